# revision 16
# baseline (speedup 1.0000x reference)
"""Trainium2 Bass kernel for nn_Attention_42288247996512 (sparse causal cross-attention).

reference:
  q = x @ Wq.T; k = cross @ Wk.T; v = x @ Wv.T
  logits = q @ k.T  (causal mask; padding mask m_q*m_k + eye > 0)
  out = softmax(logits / sqrt(128)) @ v

Sharding: 8 cores = 4 batches x 2 query-strips (SPMD). Each strip is 8 query
blocks (128 rows) grouped into 4 pairs of adjacent blocks.

Two structural optimizations vs a vanilla flash-style kernel:

1) Reassociation:  attn @ (x @ Wv.T) == (attn @ x) @ Wv.T.  Each core owns
   1024 query rows but would need all 2048 key rows of v, so projecting
   t = attn@x (1024 cols) instead of v (2048 rows) halves that matmul.

2) Key compaction: ~half the keys are padding-masked (exp == 0 columns).
   The kernel is JIT-specialized on the mask's *structure*: keys are
   host-compacted to the active ones, shrinking kT/logits/exp/transpose/AX
   nearly 2x.  Masked queries (whose softmax row is a delta at the diagonal,
   so out[q] = v[q]) bypass attention entirely: x.T is DMA'd into the t
   buffer and the attention results are merged over it with predicated
   copies (mask = query-unmasked), then t @ Wv.T produces v[q] for them
   directly.  The structure parameters are recomputed from the input mask on
   every call (and cached), so the kernel stays correct for any input.

All streamed operands are bf16; PSUM accumulation is f32.  Host does layout
packs/bf16 casts, gathers, additive-mask building, and the final denominator
divide + scatter (as in the baseline kernel).
"""
import math
import threading

import ml_dtypes
import numpy as np

B, S, D, DA = 4, 2048, 1024, 128
P = 128
NCORES = 8
BIG = 32768.0  # power of two: exactly representable in bf16
NQ = 1024      # query rows per core strip
KC = D // P    # 8 contraction chunks of 128

# strips: pairs of adjacent blocks; block g attends orig keys < (g+1)*128
STRIPS = [
    [0, 1, 14, 15, 6, 7, 8, 9],
    [2, 3, 12, 13, 4, 5, 10, 11],
]

_BUILD_LOCK = threading.Lock()
_CACHE: dict = {}


def _derive_params(mask_f):
    """Compute the SPMD kernel structure (max over all 8 cores) from the mask.

    Returns a hashable params tuple:
      nkb:   compacted key blocks (128 each)
      kb:    per-pair key-block count (AX contraction length)
      chunks: per-pair tuple of chunk widths (<=512, multiples of 128)
      masked: per-pair tuple of bools - does chunk j need an additive mask
    """
    nkb = 0
    kb = [0, 0, 0, 0]
    for b in range(B):
        m = mask_f[b] > 0
        ck = np.cumsum(m)          # ck[s] = # active keys <= s
        nk = int(ck[-1])
        nkb = max(nkb, (nk + P - 1) // P)
        for p in range(2):
            blocks = STRIPS[p]
            for pr in range(4):
                g = max(blocks[2 * pr], blocks[2 * pr + 1])
                bmax = int(ck[(g + 1) * P - 1])
                kb[pr] = max(kb[pr], (bmax + P - 1) // P)
    chunks = []
    for pr in range(4):
        w = kb[pr] * P
        ch = []
        while w > 0:
            ch.append(min(512, w))
            w -= min(512, w)
        chunks.append(tuple(ch))
    # chunk (pr, j) needs a mask iff for ANY core its key range reaches
    # beyond that core's (min unmasked-row boundary) or active-key count
    masked = [[False] * len(chunks[pr]) for pr in range(4)]
    for b in range(B):
        m = mask_f[b] > 0
        ck = np.cumsum(m)
        nk = int(ck[-1])
        for p in range(2):
            blocks = STRIPS[p]
            for pr in range(4):
                rows = np.concatenate(
                    [np.arange(g * P, (g + 1) * P)
                     for g in (blocks[2 * pr], blocks[2 * pr + 1])])
                urows = rows[m[rows]]
                bmin = int(ck[urows].min()) if len(urows) else 0
                base = 0
                for j, w in enumerate(chunks[pr]):
                    if base + w > bmin or base + w > nk:
                        masked[pr][j] = True
                    base += w
    return (nkb, tuple(kb), tuple(chunks),
            tuple(tuple(mj) for mj in masked))


def _build(params):
    from contextlib import ExitStack

    import concourse.bass as bass
    import concourse.mybir as mybir
    import concourse.tile as tile
    from concourse import bacc
    from concourse.masks import make_identity

    nkb, KB, CHUNKS, MASKED = params
    NK = max(nkb, 1) * P  # padded compacted key width
    nt = sum(2 * sum(mj) for mj in MASKED)  # dmask tile count
    NT = max(nt, 1)

    dt = mybir.dt
    f32 = dt.float32
    bf16 = dt.bfloat16
    AF = mybir.ActivationFunctionType
    ALU = mybir.AluOpType

    nc = bacc.Bacc("TRN2", target_bir_lowering=False, debug=False)

    # DRAM inputs (bf16 unless noted); weights host-packed to [P, ...] so
    # DMA rows are contiguous >=512B runs.
    xkc = nc.dram_tensor("xkc", [NK, D], bf16, kind="ExternalInput").ap()
    cTc = nc.dram_tensor("cTc", [D, NK], bf16, kind="ExternalInput").ap()
    xqT = nc.dram_tensor("xqT", [D, NQ], bf16, kind="ExternalInput").ap()
    wqp = nc.dram_tensor("wqp", [P, KC, DA], bf16, kind="ExternalInput").ap()
    wkp = nc.dram_tensor("wkp", [P, KC, DA], bf16, kind="ExternalInput").ap()
    wvp = nc.dram_tensor("wvp", [P, KC, D], bf16, kind="ExternalInput").ap()
    qmn = nc.dram_tensor("qmn", [P, 8], f32, kind="ExternalInput").ap()
    dm2 = nc.dram_tensor("dm2", [NT, P, 512], bf16, kind="ExternalInput").ap()
    u8 = dt.uint8
    pmask = nc.dram_tensor("pmask", [P, 4, 256], u8,
                           kind="ExternalInput").ap()

    outT = nc.dram_tensor("outT", [D, NQ], bf16,
                          kind="ExternalOutput").ap()
    den = nc.dram_tensor("den", [P, 8], f32, kind="ExternalOutput").ap()

    xkc_r = xkc.rearrange("(kb p) d -> p kb d", p=P)
    cTc_r = cTc.rearrange("(kc p) s -> p kc s", p=P)
    xqT_r = xqT.rearrange("(kc p) q -> p kc q", p=P)
    outT_r = outT.rearrange("(dmc p) q -> p dmc q", p=P)

    # dmask tile index for (pair, chunk, blk): assigned lazily in emission
    # order so the DMA (split in two) streams tiles in first-use order; the
    # host builds dm2 in this same order (read back via _CACHE["dm_order"]).
    dmidx = {}

    def dm_tile(pr, j, blk):
        key = (pr, j, blk)
        if key not in dmidx:
            dmidx[key] = len(dmidx)
        return dmidx[key]

    # kT chunk layout over NK cols
    kt_chunks = []
    w = NK
    while w > 0:
        kt_chunks.append(min(512, w))
        w -= min(512, w)

    with tile.TileContext(nc) as tc, ExitStack() as ctx:
        const = ctx.enter_context(tc.tile_pool(name="const", bufs=1))
        persist = ctx.enter_context(tc.tile_pool(name="persist", bufs=1))
        stream = ctx.enter_context(tc.tile_pool(name="stream", bufs=2))
        apool = ctx.enter_context(tc.tile_pool(name="apool", bufs=4))
        epool = ctx.enter_context(tc.tile_pool(name="epool", bufs=24))

        ident_f32 = const.tile([P, P], f32, name="ident_f32")
        make_identity(nc, ident_f32)
        ident = const.tile([P, P], bf16, name="ident")
        nc.vector.tensor_copy(ident[:], ident_f32[:])

        wq_sb = const.tile([P, KC, DA], bf16, name="wq_sb")
        wk_sb = const.tile([P, KC, DA], bf16, name="wk_sb")
        wv_sb = const.tile([P, KC, D], bf16, name="wv_sb")
        qmn_sb = const.tile([P, 8], f32, name="qmn_sb")
        dm_sb = const.tile([P, NT, 512], bf16, name="dm_sb")
        pm_sb = const.tile([P, 4, 256], u8, name="pm_sb")

        kT_sb = persist.tile([P, NK], bf16, name="kT_sb")
        qT_sb = persist.tile([P, NQ], bf16, name="qT_sb")
        xk_sb = persist.tile([P, max(nkb, 1), D], bf16, name="xk_sb")
        tT_sb = persist.tile([P, KC, NQ], bf16, name="tT_sb")
        den_sb = persist.tile([P, 8], f32, name="den_sb")

        eTs_all = {pr: [] for pr in range(4)}
        daccs_all = {pr: [[], []] for pr in range(4)}

        # PSUM: 4 pools x 2 bufs x 2KB = all 8 banks.
        psl_pool = ctx.enter_context(
            tc.tile_pool(name="psl", bufs=2, space="PSUM"))
        psT_pool = ctx.enter_context(
            tc.tile_pool(name="psT", bufs=2, space="PSUM"))
        psax_pool = ctx.enter_context(
            tc.tile_pool(name="psax", bufs=2, space="PSUM"))
        psw_pool = ctx.enter_context(
            tc.tile_pool(name="psw", bufs=2, space="PSUM"))

        # ---- projections (DMA emission order == SP FIFO delivery order) ----
        def kT_chunk(j):
            if nkb == 0:
                return
            w = kt_chunks[j]
            base = sum(kt_chunks[:j])
            ctj = stream.tile([P, KC, 512], bf16, tag="ct", name=f"ct{j}",
                              bufs=2)
            nc.sync.dma_start(ctj[:, :, :w], cTc_r[:, :, base:base + w])
            ps_k = psax_pool.tile([P, 512], f32, tag="psax", name=f"ps_k{j}")
            for kc in range(KC):
                nc.tensor.matmul(
                    ps_k[:, :w],
                    lhsT=wk_sb[:, kc, :],
                    rhs=ctj[:, kc, :w],
                    start=(kc == 0), stop=(kc == KC - 1),
                )
            nc.any.tensor_copy(kT_sb[:, base:base + w], ps_k[:, :w])

        # PE warmup: dependency-free matmuls on the identity keep the PE busy
        # (and ramp its p-state to full clock) while the first input DMAs
        # stream in; each is only ~120ns so real work is barely delayed.
        for wu in range(48):
            pswu = psw_pool.tile([P, P], f32, tag="psw", name=f"pswu{wu}",
                                 padded_shape=[P, 512])
            nc.tensor.matmul(pswu[:], lhsT=ident[:], rhs=ident[:],
                             start=True, stop=True)

        # t.T is pre-filled with x_strip.T: it doubles as the qT projection
        # rhs AND as the pass-through giving masked queries out[q] = v[q]
        # (attention results are merged over it with predicated copies).
        nc.sync.dma_start(wq_sb[:], wqp)
        ps_q = [psl_pool.tile([P, 512], f32, tag="psl", name=f"ps_q{n}")
                for n in range(2)]

        def qT_fill(n, k0, k1):
            nc.sync.dma_start(tT_sb[:, k0:k1, n * 512:(n + 1) * 512],
                              xqT_r[:, k0:k1, n * 512:(n + 1) * 512])

        def qT_mm(n, k0, k1):
            for kc in range(k0, k1):
                nc.tensor.matmul(
                    ps_q[n][:],
                    lhsT=wq_sb[:, kc, :],
                    rhs=tT_sb[:, kc, n * 512:(n + 1) * 512],
                    start=(kc == 0), stop=(kc == KC - 1),
                )

        # all tT fills are emitted before their first reader so the fill
        # DMAs never serialize against qT matmul reads (WAR)
        qT_fill(0, 0, KC)
        qT_mm(0, 0, KC)
        nc.sync.dma_start(wk_sb[:], wkp)
        kT_chunk(0)
        qT_fill(1, 0, 4)
        qT_mm(1, 0, 4)
        qT_fill(1, 4, KC)
        qT_mm(1, 4, KC)
        for n in range(2):
            nc.any.tensor_copy(qT_sb[:, n * 512:(n + 1) * 512], ps_q[n][:])
        nc.sync.dma_start(qmn_sb[:], qmn[:])
        NTA = min(6, NT)
        nc.sync.dma_start(dm_sb[:, 0:NTA, :],
                          dm2.rearrange("t p w -> p t w")[:, 0:NTA, :])
        nc.sync.dma_start(pm_sb[:], pmask)

        # ---- attention stages ----
        def stage_a_chunk(pr, j, mid_hook=None):
            w = CHUNKS[pr][j]
            base = sum(CHUNKS[pr][:j])
            nks = (w + P - 1) // P
            psTs = [psT_pool.tile([P, 256], bf16, tag="psT",
                                  name=f"psT{pr}_{j}_{ks}",
                                  padded_shape=[P, 1024])
                    for ks in range(nks)]
            es = []
            for blk in range(2):
                slot = pr * 2 + blk
                psl = psl_pool.tile([P, 512], f32, tag="psl",
                                    name=f"psl{slot}_{j}")
                nc.tensor.matmul(
                    psl[:, :w],
                    lhsT=qT_sb[:, slot * P:(slot + 1) * P],
                    rhs=kT_sb[:, base:base + w],
                    start=True, stop=True,
                )
                dac = apool.tile([P, 1], f32, tag="dac",
                                 name=f"dac{slot}_{j}", bufs=12)
                e = apool.tile([P, 512], bf16, tag="e", name=f"e{slot}_{j}")
                if MASKED[pr][j]:
                    sbl = apool.tile([P, 512], f32, tag="sbl",
                                     name=f"sbl{slot}_{j}")
                    nc.vector.tensor_tensor(
                        out=sbl[:, :w], in0=psl[:, :w],
                        in1=dm_sb[:, dm_tile(pr, j, blk), :w], op=ALU.add)
                    src = sbl
                else:
                    src = psl
                nc.scalar.activation(
                    e[:, :w], src[:, :w], AF.Exp,
                    bias=qmn_sb[:, slot:slot + 1], scale=1.0,
                    accum_out=dac[:],
                )
                daccs_all[pr][blk].append(dac)
                es.append(e)
            if mid_hook is not None:
                mid_hook()
            for blk in range(2):
                for ks in range(nks):
                    nc.tensor.transpose(
                        psTs[ks][:, blk * P:(blk + 1) * P],
                        es[blk][:, ks * P:(ks + 1) * P],
                        ident[:],
                    )
            for ks in range(nks):
                eT = epool.tile([P, 256], bf16, tag="eT",
                                name=f"eT{pr}_{j}_{ks}")
                nc.any.tensor_copy(eT[:], psTs[ks][:])
                eTs_all[pr].append(eT)

        def stage_den(pr):
            for blk in range(2):
                slot = pr * 2 + blk
                dl = daccs_all[pr][blk]
                dst = den_sb[:, slot:slot + 1]
                if len(dl) == 1:
                    nc.any.tensor_copy(dst, dl[0][:])
                else:
                    nc.vector.tensor_tensor(
                        out=dst, in0=dl[0][:], in1=dl[1][:], op=ALU.add)
                    for d in dl[2:]:
                        nc.vector.tensor_tensor(
                            out=dst, in0=dst, in1=d[:], op=ALU.add)

        def stage_ax(pr, mid_hook=None):
            # t.T[xd, q] = sum_kb x[kb].T-contraction with eT over the
            # pair's compacted key blocks; merged into the pass-through
            # with a predicated copy (pmask: 1 = query unmasked).
            stage_den(pr)
            eTs = eTs_all[pr]
            for xdc in range(KC):
                psax = psax_pool.tile([P, 256], f32, tag="psax",
                                      name=f"psax{pr}_{xdc}",
                                      padded_shape=[P, 512])
                for kb in range(KB[pr]):
                    nc.tensor.matmul(
                        psax[:],
                        lhsT=xk_sb[:, kb, xdc * P:(xdc + 1) * P],
                        rhs=eTs[kb][:],
                        start=(kb == 0), stop=(kb == KB[pr] - 1),
                    )
                nc.vector.copy_predicated(
                    tT_sb[:, xdc, pr * 256:(pr + 1) * 256],
                    pm_sb[:, pr, :], psax[:])
                if mid_hook is not None and xdc == 3:
                    mid_hook()

        _osbs = {}

        def stage_twv(pr, dm_half=None, den_dma=False):
            # out.T[dm, q] = Wv.T.T @ t.T for this pair's 256 query columns
            halves = [0, 1] if dm_half is None else [dm_half]
            osb = _osbs.setdefault(
                pr, apool.tile([P, KC, 256], bf16, tag="osb",
                               name=f"osb{pr}", bufs=2))
            if den_dma:
                nc.scalar.dma_start(den[:], den_sb[:])
            fine = den_dma  # last pair: drain per-dmc so the tail is short
            for h in halves:
                for dmc in range(4 * h, 4 * h + 4):
                    psw = psw_pool.tile([P, 256], f32, tag="psw",
                                        name=f"psw{pr}_{dmc}",
                                        padded_shape=[P, 512])
                    for xdc in range(KC):
                        nc.tensor.matmul(
                            psw[:],
                            lhsT=wv_sb[:, xdc, dmc * P:(dmc + 1) * P],
                            rhs=tT_sb[:, xdc, pr * 256:(pr + 1) * 256],
                            start=(xdc == 0), stop=(xdc == KC - 1),
                        )
                    # alternate engines so the final copies drain in parallel
                    if fine or dmc % 2 == 0:
                        nc.vector.tensor_copy(osb[:, dmc, :], psw[:])
                    else:
                        nc.scalar.copy(osb[:, dmc, :], psw[:])
                    if fine and h == 1:
                        nc.sync.dma_start(
                            outT_r[:, dmc:dmc + 1,
                                   pr * 256:(pr + 1) * 256],
                            osb[:, dmc:dmc + 1, :])
                if not (fine and h == 1):
                    nc.scalar.dma_start(
                        outT_r[:, 4 * h:4 * h + 4, pr * 256:(pr + 1) * 256],
                        osb[:, 4 * h:4 * h + 4, :])

        def A(pr, j, mid_hook=None):
            if j < len(CHUNKS[pr]) and CHUNKS[pr][j] > 0:
                stage_a_chunk(pr, j, mid_hook=mid_hook)
            elif mid_hook is not None:
                mid_hook()

        def AX(pr, mid_hook=None):
            if KB[pr] > 0:
                stage_ax(pr, mid_hook=mid_hook)
            elif mid_hook is not None:
                mid_hook()

        # ---- fused schedule (PE emission order tuned to DMA arrivals) ----
        nch = [len(CHUNKS[pr]) for pr in range(4)]
        ka = min(KB[0], nkb)
        kbb = min(max(KB[0], KB[2]), nkb)
        A(0, 0)
        # A(1,0) is the mask-free chunk: cheap PE filler while masks stream
        if nch[1] > 0:
            A(1, 0, mid_hook=lambda: kT_chunk(1)
              if len(kt_chunks) > 1 else None)
        if nch[1] > 1:
            A(1, 1)
        if nkb and ka:
            nc.sync.dma_start(xk_sb[:, 0:ka, :], xkc_r[:, 0:ka, :])
        AX(0)
        nc.sync.dma_start(wv_sb[:, :, 0:512], wvp[:, :, 0:512])
        if NT > NTA:
            nc.sync.dma_start(dm_sb[:, NTA:NT, :],
                              dm2.rearrange("t p w -> p t w")[:, NTA:NT, :])
        stage_twv(0, dm_half=0)
        A(2, 0, mid_hook=lambda: [kT_chunk(j) for j in
                                  range(2, len(kt_chunks))])
        A(2, 1)
        if nkb and kbb > ka:
            nc.sync.dma_start(xk_sb[:, ka:kbb, :], xkc_r[:, ka:kbb, :])
        AX(2)
        for j in range(2, nch[1]):
            A(1, j)
        if nkb and nkb > kbb:
            nc.sync.dma_start(xk_sb[:, kbb:nkb, :], xkc_r[:, kbb:nkb, :])
        nc.sync.dma_start(wv_sb[:, :, 512:1024], wvp[:, :, 512:1024])
        A(3, 0)
        AX(1)
        stage_twv(0, dm_half=1)
        stage_twv(2, dm_half=0)
        A(3, 1)
        stage_twv(2, dm_half=1)
        for j in range(2, nch[3]):
            A(3, j)
        AX(3)
        stage_twv(1)
        stage_twv(3, den_dma=True)

    nc.compile()
    return nc, dmidx


def _get_nc(params=None):
    with _BUILD_LOCK:
        if params is None:
            # harness/test introspection path: last-built (or default) kernel
            if "nc" in _CACHE:
                return _CACHE["nc"]
            params = _CACHE.get("params")
            if params is None:
                raise RuntimeError("call kernel() first to JIT the program")
        if _CACHE.get("params") != params or "nc" not in _CACHE:
            _CACHE["params"] = params
            _CACHE["nc"], _CACHE["dm_order"] = _build(params)
        return _CACHE["nc"]


def kernel(x, cross, Wq, Wk, Wv, mask):
    from concourse import bass_utils

    bf = ml_dtypes.bfloat16
    x = np.asarray(x, dtype=np.float32)
    cross = np.asarray(cross, dtype=np.float32)
    scale = 1.0 / math.sqrt(DA)
    mf = np.asarray(mask).astype(np.float32)  # [B, S]

    params = _derive_params(mf)
    nc = _get_nc(params)
    nkb, KB, CHUNKS, MASKED = params
    NK = max(nkb, 1) * P
    NT = max(sum(2 * sum(mj) for mj in MASKED), 1)

    def pack_w(wT, m_cols):
        # [D, m] -> [P, KC, m] with [p, kc, m] = wT[kc*128 + p, m]
        return np.ascontiguousarray(
            wT.reshape(KC, P, m_cols).transpose(1, 0, 2)).astype(bf)

    wqp_h = pack_w((np.asarray(Wq, np.float32) * scale).T, DA)
    wkp_h = pack_w(np.asarray(Wk, np.float32).T, DA)
    wvp_h = pack_w(np.asarray(Wv, np.float32).T, D)

    in_maps = []
    rows_per_core = []
    for core in range(NCORES):
        b, p = divmod(core, 2)
        blocks = STRIPS[p]
        rows = np.concatenate([np.arange(g * P, (g + 1) * P) for g in blocks])
        mb = mf[b] > 0
        ck = np.cumsum(mb)           # active keys <= s
        active = np.nonzero(mb)[0]   # orig idx of compacted keys
        nk = len(active)
        rows_per_core.append((b, rows, mb[rows]))
        # compacted key-side tensors (zero pad to NK)
        xkc_h = np.zeros((NK, D), np.float32)
        xkc_h[:nk] = x[b][active]
        cTc_h = np.zeros((D, NK), np.float32)
        cTc_h[:, :nk] = cross[b].T[:, active]
        mq = mb[rows]
        qmn_h = np.ascontiguousarray(
            (-BIG * (1.0 - mq.astype(np.float32))).reshape(8, P).T)
        # additive causal/pad masks in compacted key coords, per masked chunk
        dm_h = np.full((NT, P, 512), -BIG, np.float32)
        ck_rows = ck[rows]  # allowed-key count per strip row
        for (pr, j, blk), ti in _CACHE["dm_order"].items():
            w = CHUNKS[pr][j]
            base = sum(CHUNKS[pr][:j])
            ckb = ck_rows[(pr * 2 + blk) * P:(pr * 2 + blk + 1) * P]
            kidx = base + np.arange(w)
            dm_h[ti, :, :w] = np.where(
                kidx[None, :] < ckb[:, None], 0.0, -BIG)
        # predication mask: 1 = query unmasked (take AX result)
        pm_h = np.broadcast_to(
            mq.astype(np.float32).reshape(4, 256)[None, :, :], (P, 4, 256))
        in_maps.append({
            "xkc": xkc_h.astype(bf),
            "cTc": cTc_h.astype(bf),
            "xqT": np.ascontiguousarray(x[b][rows].T).astype(bf),
            "wqp": wqp_h,
            "wkp": wkp_h,
            "wvp": wvp_h,
            "qmn": qmn_h,
            "dm2": dm_h.astype(bf),
            "pmask": np.ascontiguousarray(pm_h).astype(np.uint8),
        })

    _CACHE["in_maps"] = in_maps
    res = bass_utils.run_bass_kernel_spmd(
        nc, in_maps, core_ids=list(range(NCORES)))

    out = np.empty((B, S, D), np.float32)
    for core in range(NCORES):
        b, rows, mq = rows_per_core[core]
        r = res.results[core]
        o = r["outT"].T.astype(np.float32)  # [1024 q, 1024 dm]
        denf = r["den"].T.reshape(-1)  # [1024] strip-ordered
        denf = np.where(mq, denf, 1.0)  # masked queries: out = v[q] directly
        out[b, rows] = o / denf[:, None]
    return out
